# revision 3
# baseline (speedup 1.0000x reference)
"""Gaussian 2x2 splat (DifferentiableSquareSensor) on 8 Trainium2 NeuronCores.

Full inputs in, full 1024x1024 image out.

Math: x,y are uniform in [0,1), so pixel coords land in [512,1024) and with
sigma=0.1 every Gaussian tap outside the nearest 2x2 neighborhood is <= e^-50
-- invisible in fp32.  The normalized per-axis weight pair collapses to a
sigmoid:  gx0/(gx0+gx1) = sigmoid(50*(1-2*xf)),  since xf + (1-xf) = 1.

Device algorithm (per core = one 64-column strip of the active 512x512):
  Each point makes two "row visits" (rows ry, ry+1).  A visit deposits
  w0 = v*ny*nx0 into (r, cx) and w1 = v*ny*nx1 into (r, cx+1) where
  ny = sigmoid(+-50(1-2yf)), nx_i = sigmoid(+-50(1-2xf)).
  The host sorts visits per SBUF partition (p = row & 127) into 260
  (row-chunk, column) buckets laid out as segments on a slot grid such
  that segment s contains grid slot s*G.  The device computes the weights
  (3 ACT sigmoids + 3 Pool multiplies), then forward and backward
  segmented prefix scans (DVE) of W0/W1; the segment total is recovered at
  the static grid slots as F[sG] + B[sG] - W[sG].  The host adds the two
  per-pixel partial sums (w0 of column c, w1 of column c-1) while
  reassembling the image.
"""

import json
import os
import sys

import numpy as np

for _p in ("/opt/trn_rl_repo", "/root/.axon_site/_ro/trn_rl_repo"):
    if os.path.isdir(_p) and _p not in sys.path:
        sys.path.append(_p)

import concourse.bass as bass
import concourse.mybir as mybir
from concourse.bass_utils import run_bass_kernel_spmd
from concourse.tile import TileContext

P = 128
NCORES = 8
NROWCHUNK = 4            # 512 rows / 128 partitions
NCX = 65                 # cx values per core incl. left-neighbor duplicate col
NBUCK = NROWCHUNK * NCX  # 260 buckets per partition
NCH = 4                  # device chunks (scan reset boundaries)
CELLS_PER_CHUNK = 68     # grid cells per chunk (>= 65 buckets + spares)
NSEG = NCH * CELLS_PER_CHUNK  # 272 extraction cells
F32 = mybir.dt.float32
F16 = mybir.dt.float16


def _split_multiwait(nc):
    """This walrus build rejects >1 sync-wait per instruction; split extras
    into single-wait NoOps placed immediately before on the same engine."""
    orig = nc.to_json_bytes

    def patched():
        js = json.loads(orig().decode())
        for fn in js["functions"]:
            for blk in fn["blocks"]:
                newlist = []
                for inst in blk["instructions"]:
                    si = inst.get("sync_info")
                    ow = (si or {}).get("on_wait") or []
                    if len(ow) > 1:
                        for k, w in enumerate(ow[:-1]):
                            newlist.append({
                                "name": f"{inst['name']}-w{k}",
                                "opcode": "NoOp",
                                "engine": inst["engine"],
                                "ins": [], "outs": [],
                                "sync_info": {"on_wait": [w], "on_update": []},
                                "bass_nofuse": True,
                            })
                        si["on_wait"] = [ow[-1]]
                    newlist.append(inst)
                blk["instructions"] = newlist
        return json.dumps(js).encode()

    nc.to_json_bytes = patched


def _build_module(g):
    """Device module for grid pitch g: NS = NSEG*g slots per partition."""
    L = CELLS_PER_CHUNK * g      # slots per chunk
    NS = NCH * L
    nc = bass.Bass("TRN2", target_bir_lowering=False, debug=False,
                   num_devices=NCORES)
    ux_d = nc.dram_tensor("ux", [P, NS], F32, kind="ExternalInput")
    uy_d = nc.dram_tensor("uy", [P, NS], F32, kind="ExternalInput")
    v_d = nc.dram_tensor("v", [P, NS], F16, kind="ExternalInput")
    m_d = nc.dram_tensor("m", [P, NS + 1], F16, kind="ExternalInput")
    q0_d = nc.dram_tensor("q0", [P, NSEG], F32, kind="ExternalOutput")
    q1_d = nc.dram_tensor("q1", [P, NSEG], F32, kind="ExternalOutput")

    AOP = mybir.AluOpType

    with TileContext(nc) as tc:
        with (
            tc.tile_pool(name="pers", bufs=1) as pers,
            tc.tile_pool(name="chk", bufs=2) as chk,
        ):
            Q0 = pers.tile([P, NSEG], F32)
            Q1 = pers.tile([P, NSEG], F32)

            for c in range(NCH):
                sl = slice(c * L, (c + 1) * L)
                UX = chk.tile([P, L], F32, name="UX")
                UY = chk.tile([P, L], F32, name="UY")
                V = chk.tile([P, L], F16, name="V")
                M = chk.tile([P, L + 1], F16, name="M")
                nc.sync.dma_start(UX[:], ux_d[:, sl])
                nc.sync.dma_start(V[:], v_d[:, sl])
                nc.scalar.dma_start(UY[:], uy_d[:, sl])
                nc.scalar.dma_start(M[:], m_d[:, c * L:(c + 1) * L + 1])

                NX0 = chk.tile([P, L], F32, name="NX0")
                nc.scalar.activation(NX0[:], UX[:],
                                     mybir.ActivationFunctionType.Sigmoid,
                                     bias=0.0, scale=1.0)
                NX1 = chk.tile([P, L], F32, name="NX1")
                nc.scalar.activation(NX1[:], UX[:],
                                     mybir.ActivationFunctionType.Sigmoid,
                                     bias=0.0, scale=-1.0)
                NY = chk.tile([P, L], F32, name="NY")
                nc.scalar.activation(NY[:], UY[:],
                                     mybir.ActivationFunctionType.Sigmoid,
                                     bias=0.0, scale=1.0)

                QV = chk.tile([P, L], F32, name="QV")
                nc.gpsimd.tensor_tensor(out=QV[:], in0=V[:], in1=NY[:],
                                        op=AOP.mult)
                W0 = chk.tile([P, L], F32, name="W0")
                nc.gpsimd.tensor_tensor(out=W0[:], in0=QV[:], in1=NX0[:],
                                        op=AOP.mult)
                W1 = chk.tile([P, L], F32, name="W1")
                nc.gpsimd.tensor_tensor(out=W1[:], in0=QV[:], in1=NX1[:],
                                        op=AOP.mult)

                def rev(t, off=0):
                    ap = t[:]
                    return bass.AP(ap.tensor, ap.offset + off + (L - 1),
                                   [ap.ap[0], [-1, L]])

                def grid(t):
                    ap = t[:]
                    return bass.AP(ap.tensor, ap.offset,
                                   [ap.ap[0], [g, CELLS_PER_CHUNK]])

                MF = M[:, 0:L]
                F0 = chk.tile([P, L], F32, name="F0")
                nc.vector.tensor_tensor_scan(
                    out=F0[:], data0=MF, data1=W0[:], initial=0.0,
                    op0=AOP.mult, op1=AOP.add)
                B0 = chk.tile([P, L], F32, name="B0")
                nc.vector.tensor_tensor_scan(
                    out=rev(B0), data0=rev(M, off=1), data1=rev(W0),
                    initial=0.0, op0=AOP.mult, op1=AOP.add)
                F1 = chk.tile([P, L], F32, name="F1")
                nc.vector.tensor_tensor_scan(
                    out=F1[:], data0=MF, data1=W1[:], initial=0.0,
                    op0=AOP.mult, op1=AOP.add)
                B1 = chk.tile([P, L], F32, name="B1")
                nc.vector.tensor_tensor_scan(
                    out=rev(B1), data0=rev(M, off=1), data1=rev(W1),
                    initial=0.0, op0=AOP.mult, op1=AOP.add)

                qsl = slice(c * CELLS_PER_CHUNK, (c + 1) * CELLS_PER_CHUNK)
                T0 = chk.tile([P, CELLS_PER_CHUNK], F32, name="T0")
                nc.vector.tensor_tensor(out=T0[:], in0=grid(F0), in1=grid(B0),
                                        op=AOP.add)
                nc.vector.tensor_tensor(out=Q0[:, qsl], in0=T0[:],
                                        in1=grid(W0), op=AOP.subtract)
                T1 = chk.tile([P, CELLS_PER_CHUNK], F32, name="T1")
                nc.vector.tensor_tensor(out=T1[:], in0=grid(F1), in1=grid(B1),
                                        op=AOP.add)
                nc.vector.tensor_tensor(out=Q1[:, qsl], in0=T1[:],
                                        in1=grid(W1), op=AOP.subtract)

            nc.sync.dma_start(q0_d[:], Q0[:])
            nc.sync.dma_start(q1_d[:], Q1[:])

    _split_multiwait(nc)
    return nc


def _layout(counts, g):
    """Greedy grid layout per stream row.

    counts: [R, NBUCK] visit counts.  Returns (S, cellof) int64 [R, NBUCK]:
    global start slot of each bucket's segment and the extraction cell it
    owns, or None if infeasible at this g.
    """
    R = counts.shape[0]
    L = CELLS_PER_CHUNK * g
    rows = np.arange(R)
    # rank buckets by count desc; deal round-robin into chunks
    rankdesc = np.argsort(-counts, axis=1, kind="stable")
    S = np.zeros((R, NBUCK), np.int64)
    cellof = np.zeros((R, NBUCK), np.int64)
    for ch in range(NCH):
        bseq = rankdesc[:, ch::NCH]                    # [R, 65] desc
        nb = bseq.shape[1]
        csort = np.take_along_axis(counts, bseq, axis=1).astype(np.int64)
        hi = np.zeros(R, np.int64)
        lo = np.full(R, nb - 1, dtype=np.int64)
        t = np.zeros(R, np.int64)
        gcell = np.zeros(R, np.int64)
        for j in range(nb):
            c_hi = csort[rows, hi]
            t_hi = np.maximum(t + c_hi, gcell * g + 1)
            g_hi = (t_hi - 1) // g + 1
            ok = g_hi <= CELLS_PER_CHUNK - (nb - 1 - j)
            pick = np.where(ok, hi, lo)
            b = bseq[rows, pick]
            cb = csort[rows, pick]
            S[rows, b] = ch * L + t
            cellof[rows, b] = ch * CELLS_PER_CHUNK + gcell
            t = np.maximum(t + cb, gcell * g + 1)
            gcell = (t - 1) // g + 1
            hi = hi + ok
            lo = lo - (~ok)
            if not ((t <= gcell * g).all()):
                return None, None
        if (t > L).any() or (gcell > CELLS_PER_CHUNK).any():
            return None, None
    return S, cellof


def _prep(x, y, v, g):
    """Build per-core device inputs + assembly indices."""
    one = np.float32(1.0)
    xp = (x + one) * np.float32(512.0)
    yp = (y + one) * np.float32(512.0)
    xi = np.floor(xp).astype(np.int32)
    yi = np.floor(yp).astype(np.int32)
    xf = xp - xi
    yf = yp - yi
    cx = xi - 512
    ry = yi - 512
    ux = np.float32(50.0) * (one - np.float32(2.0) * xf)
    uy = np.float32(50.0) * (one - np.float32(2.0) * yf)

    mB = ry < 511
    r_a = np.concatenate([ry, ry[mB] + 1])
    uy_a = np.concatenate([uy, -uy[mB]])
    ux_a = np.concatenate([ux, ux[mB]])
    v_a = np.concatenate([v, v[mB]])
    cx_a = np.concatenate([cx, cx[mB]])

    core = (cx_a >> 6).astype(np.int64)
    dup = ((cx_a & 63) == 63) & (cx_a != 511)
    core_f = np.concatenate([core, core[dup] + 1])
    r_f = np.concatenate([r_a, r_a[dup]])
    ux_f = np.concatenate([ux_a, ux_a[dup]])
    uy_f = np.concatenate([uy_a, uy_a[dup]])
    v_f = np.concatenate([v_a, v_a[dup]])
    cx_f = np.concatenate([cx_a, cx_a[dup]])

    p = (r_f & 127).astype(np.int64)
    rc = (r_f >> 7).astype(np.int64)
    cxloc = cx_f - (core_f << 6) + 1
    bucket = rc * NCX + cxloc
    row = core_f * P + p                      # 0..1023
    key = row * NBUCK + bucket
    order = np.argsort(key, kind="stable")
    key_s = key[order]
    R = NCORES * P
    counts = np.bincount(key_s, minlength=R * NBUCK).reshape(R, NBUCK)

    S, cellof = _layout(counts, g)
    if S is None:
        return None

    L = CELLS_PER_CHUNK * g
    NS = NCH * L
    cflat = counts.reshape(-1)
    seg_start = np.repeat(S.reshape(-1), cflat)
    csum = np.zeros(R * NBUCK + 1, np.int64)
    np.cumsum(cflat, out=csum[1:])
    within = np.arange(key_s.size, dtype=np.int64) - np.repeat(csum[:-1], cflat)
    rows_s = key_s // NBUCK
    flat = rows_s * NS + seg_start + within

    uxa = np.zeros(R * NS, np.float32)
    uya = np.zeros(R * NS, np.float32)
    va = np.zeros(R * NS, np.float16)
    uxa[flat] = ux_f[order]
    uya[flat] = uy_f[order]
    va[flat] = v_f[order].astype(np.float16)
    ma = np.ones(R * (NS + 1), np.float16)
    midx = (np.arange(R)[:, None] * (NS + 1) + S).ravel()
    ma[midx] = 0.0
    ma.reshape(R, NS + 1)[:, NS] = 0.0

    per_core = []
    for k in range(NCORES):
        rsl = slice(k * P, (k + 1) * P)
        per_core.append({
            "ux": uxa.reshape(R, NS)[rsl],
            "uy": uya.reshape(R, NS)[rsl],
            "v": va.reshape(R, NS)[rsl],
            "m": ma.reshape(R, NS + 1)[rsl],
        })
    return per_core, cellof.reshape(NCORES, P, NBUCK)


_CACHE = {}


def kernel(x, y, values):
    x = np.asarray(x, dtype=np.float32)
    y = np.asarray(y, dtype=np.float32)
    v = np.asarray(values, dtype=np.float32)

    g = 18
    prep = _prep(x, y, v, g)
    while prep is None:
        g += 2
        prep = _prep(x, y, v, g)
    per_core, cellof = prep

    if g not in _CACHE:
        _CACHE[g] = _build_module(g)
    nc = _CACHE[g]

    res = run_bass_kernel_spmd(nc, per_core, core_ids=list(range(NCORES)))

    img = np.zeros((1024, 1024), dtype=np.float32)
    prow = np.arange(P)[:, None]
    for k in range(NCORES):
        q0 = res.results[k]["q0"]            # [P, NSEG]
        q1 = res.results[k]["q1"]
        q0b = q0[prow, cellof[k]].reshape(P, NROWCHUNK, NCX)
        q1b = q1[prow, cellof[k]].reshape(P, NROWCHUNK, NCX)
        pix = q0b[:, :, 1:] + q1b[:, :, :64]          # [P, 4, 64]
        for rc in range(NROWCHUNK):
            img[512 + rc * P:512 + (rc + 1) * P,
                512 + 64 * k:512 + 64 * (k + 1)] = pix[:, rc, :]
    return img
